# revision 2
# baseline (speedup 1.0000x reference)
"""BatchNormSPD Trainium2 kernel (Bass/Tile), v4: GPSIMD-free hot path.

Pipeline (per 64x64 SPD matrix, 4096 total, 512/core on 8 cores):
  Xp = sqrt(X)                  monomial-PS poly (deg 8, s=3) in fp16
  A1 = R1 Xp R1t, T1 = log(A1)  congruence + poly (deg 5)
  A2 = R2 Xp R2t, T  = log(A2)  (Karcher-mean whitening via 2 AllReduces)
  H = Qst' T Qst, E = exp(H)    poly (deg 5)
  Z = Pmt' E Pmt, Y = Z @ Z
fp16 64x64 quadrant-pair matmuls (pair layout: top matrix in partitions
0-63, bottom in 64-127), fp32 PSUM. ALL polynomial coefficient terms
(identity + A1 + A2 per Horner block) accumulate in PSUM via
scaled-identity matmuls on the tensor engine (emitted FIRST with
start=True; accumulation onto quadrant-started psum reads stale state,
this order is exact). Per-chunk element-wise work is split
Vector (STT block finalize, affine, accumulators) / Scalar (PSUM->SBUF
copies, squares); the GPSIMD engine (measured ~15us per [128,1024]
tensor_scalar on hw) is only used for DMA triggers / collectives.
Chunks are emitted 2-way interleaved (generators) so the in-order
engine queues overlap; PSUM double-buffered by chunk parity.
Everything SBUF-resident; tiny shared-matrix path fp32.
"""
import math
import os

import numpy as np

import concourse.bacc as bacc
import concourse.tile as tile
from concourse import mybir
from concourse.bass_utils import run_bass_kernel_spmd
from concourse.masks import make_identity

F32 = mybir.dt.float32
F16 = mybir.dt.float16
MULT = mybir.AluOpType.mult
ADD = mybir.AluOpType.add
SUB = mybir.AluOpType.subtract

n = 64
EPS = 1e-5

CFG = dict(
    sqrt_ab=(0.44, 5.75), sqrt_deg=8,
    log1_ab=(0.53, 2.15), log1_deg=5,
    log2_ab=(0.56, 2.30), log2_deg=5,
    exp_r=0.65, exp_deg=5,
    expT_deg=6,
    tiny_tol=2e-6,
)

TINY_RANGES = dict(MW=(0.30, 3.30), Wc=(0.26, 3.45), Gx=(0.33, 3.72))


def cheb_coeffs(fn, a, b, ndeg):
    m = 8 * (ndeg + 1)
    theta = (np.arange(m) + 0.5) * np.pi / m
    x = np.cos(theta)
    xx = 0.5 * (b - a) * x + 0.5 * (b + a)
    fv = fn(xx)
    cc = np.zeros(ndeg + 1)
    for j in range(ndeg + 1):
        cc[j] = 2.0 / m * np.sum(fv * np.cos(j * theta))
    cc[0] *= 0.5
    return cc


def cheb_block_alpha(c, s):
    ndeg = len(c) - 1
    m = (ndeg + s) // s
    cc = np.zeros(m * s)
    cc[: ndeg + 1] = c
    alpha = np.zeros((m, s))
    for j in range(m - 1, 0, -1):
        alpha[j, 0] = cc[j * s]
        for r in range(1, s):
            val = 2 * cc[j * s + r]
            if j + 1 < m:
                val -= alpha[j + 1, s - r]
            alpha[j, r] = val
    alpha[0, 0] = cc[0]
    for r in range(1, s):
        alpha[0, r] = cc[r] - (0.5 * alpha[1, s - r] if m > 1 else 0.0)
    return alpha


def mono_poly(fn, a, b, deg):
    m = 8 * (deg + 2)
    u = np.cos((2 * np.arange(m) + 1) * np.pi / (2 * m))
    x = 0.5 * (b - a) * u + 0.5 * (b + a)
    V = np.polynomial.chebyshev.chebvander(u, deg)
    c, *_ = np.linalg.lstsq(V, fn(x), rcond=None)
    return np.polynomial.chebyshev.cheb2poly(c)


def drive(gens, width=2):
    """Round-robin-advance up to `width` generators; rolling admission."""
    active = []
    gens = list(gens)
    idx = 0
    while active or idx < len(gens):
        while len(active) < width and idx < len(gens):
            active.append(gens[idx])
            idx += 1
        for g in list(active):
            try:
                next(g)
            except StopIteration:
                active.remove(g)


class Emit:
    def __init__(self, nc, tc, pairs_per_core, chunk_pairs, batch_total):
        self.nc = nc
        self.tc = tc
        self.P = pairs_per_core
        self.C = chunk_pairs
        self.B = batch_total
        self.n_chunks = pairs_per_core // chunk_pairs
        self.FD = chunk_pairs * n
        self.W = pairs_per_core * n

        a, b = CFG["sqrt_ab"]
        self.sqrt_mono = mono_poly(np.sqrt, a, b, CFG["sqrt_deg"])
        self.sqrt_aff = (2.0 / (b - a), -(a + b) / (b - a))
        a, b = CFG["log1_ab"]
        self.log1_mono = mono_poly(np.log, a, b, CFG["log1_deg"])
        self.log1_aff = (2.0 / (b - a), -(a + b) / (b - a))
        a, b = CFG["log2_ab"]
        self.log2_mono = mono_poly(np.log, a, b, CFG["log2_deg"])
        self.log2_aff = (2.0 / (b - a), -(a + b) / (b - a))
        r = CFG["exp_r"]
        self.exp_mono = mono_poly(lambda u: np.exp(r * u), -1.0, 1.0,
                                  CFG["exp_deg"])
        self.expT_c = [1.0 / math.factorial(k)
                       for k in range(CFG["expT_deg"] + 1)]

        self.tiny_polys = {}
        for name, (a, b) in TINY_RANGES.items():
            for fname, fn in (("sqrt", np.sqrt),
                              ("rsqrt", lambda x: 1.0 / np.sqrt(x))):
                deg = None
                for d in range(8, 30):
                    c = cheb_coeffs(fn, a, b, d)
                    xs_ = np.linspace(a, b, 4001)
                    xh = (2 * xs_ - (a + b)) / (b - a)
                    err = np.abs(np.polynomial.chebyshev.chebval(xh, c)
                                 - fn(xs_)).max()
                    if err < CFG["tiny_tol"]:
                        deg = d
                        break
                assert deg is not None, (name, fname)
                self.tiny_polys[(name, fname)] = (
                    cheb_block_alpha(c, 5),
                    (2.0 / (b - a), -(a + b) / (b - a)))

    # ---------- helpers ----------
    def stt(self, eng, out, in0, scalar, in1, op0=MULT, op1=ADD):
        eng.scalar_tensor_tensor(out, in0, float(scalar), in1, op0, op1)

    def _bc(self, tiny, npairs):
        return tiny[:, None, :].to_broadcast((128, npairs, n))

    def scaled_identity(self, cval, tag, dtype=F32):
        t = self.cst.tile([128, n], dtype, tag=tag)
        self.nc.vector.tensor_scalar_mul(t[:], self.Ig[:], float(cval))
        return t

    def idw_tile(self, cval, tag):
        t = self.cst.tile([128, 128], F16, tag=tag)
        self.nc.vector.tensor_scalar_mul(t[:], self.I128[:], float(cval))
        return t

    def wave_ps(self, slot, which):
        return self.ps.tile([128, self.FD], F32, tag=f"ps{slot}{which}",
                            name=f"wps{slot}{which}")

    def tiny_ps(self, tag="ps0a"):
        return self.ps.tile([128, n], F32, tag=tag, name="tps")

    # ---------- wave matmuls (fp16 quadrant pairs) ----------
    def wave_mm(self, pt, lhsT, rhs, npairs=None, lhs_off=0, rhs_off=0,
                start=True, stop=True):
        nc = self.nc
        npairs = self.C if npairs is None else npairs
        for p in range(npairs):
            sl = slice(p * n, (p + 1) * n)
            ls = slice(lhs_off + p * n, lhs_off + (p + 1) * n)
            rs = slice(rhs_off + p * n, rhs_off + (p + 1) * n)
            nc.tensor.matmul(pt[0:64, sl], lhsT[0:64, ls], rhs[0:64, rs],
                             start=start, stop=stop, skip_group_check=True)
            nc.tensor.matmul(pt[64:128, sl], lhsT[64:128, ls], rhs[64:128, rs],
                             start=start, stop=stop, skip_group_check=True)

    def wave_rep(self, pt, lhsT, rep, npairs=None, lhs_off=0,
                 start=True, stop=True):
        nc = self.nc
        npairs = self.C if npairs is None else npairs
        for p in range(npairs):
            sl = slice(p * n, (p + 1) * n)
            ls = slice(lhs_off + p * n, lhs_off + (p + 1) * n)
            nc.tensor.matmul(pt[0:64, sl], lhsT[0:64, ls], rep[0:64, :],
                             start=start, stop=stop, skip_group_check=True)
            nc.tensor.matmul(pt[64:128, sl], lhsT[64:128, ls], rep[64:128, :],
                             start=start, stop=stop, skip_group_check=True)

    def shared_mm(self, pt, rep, rhs, npairs=None, rhs_off=0,
                  start=True, stop=True):
        nc = self.nc
        npairs = self.C if npairs is None else npairs
        width = npairs * n
        for h in range(0, width, 512):
            w = min(512, width - h)
            sl = slice(h, h + w)
            rs = slice(rhs_off + h, rhs_off + h + w)
            nc.tensor.matmul(pt[0:64, sl], rep[0:64, :], rhs[0:64, rs],
                             start=start, stop=stop, skip_group_check=True)
            nc.tensor.matmul(pt[64:128, sl], rep[64:128, :], rhs[64:128, rs],
                             start=start, stop=stop, skip_group_check=True)

    def id_mm(self, pt, coeff_tile, moving, npairs=None, start=False,
              stop=False):
        nc = self.nc
        npairs = self.C if npairs is None else npairs
        width = npairs * n
        for h in range(0, width, 512):
            w = min(512, width - h)
            sl = slice(h, h + w)
            nc.tensor.matmul(pt[:, sl], coeff_tile[:, :], moving[:, sl],
                             start=start, stop=stop, skip_group_check=True)

    # ---------- big-batch polynomial (monomial PS, s=3), generator ----------
    def poly_gen(self, pfx, A1, mono, out, slot):
        """Yield-granular evaluation of p(A1) into `out` ([128, FD] slice).

        All coefficient terms (c*I128 @ A1 / A2) run on the tensor engine;
        one DVE stt per Horner block moves PSUM->SBUF and adds c0*I.
        """
        v, sc = self.nc.vector, self.nc.scalar
        deg = len(mono) - 1
        s = 3
        m = (deg + s) // s
        c = np.zeros(m * s)
        c[: deg + 1] = mono
        FD = self.FD
        wk = self.wk

        psA2 = self.wave_ps(slot, "a")
        self.wave_mm(psA2, A1, A1)
        yield
        A2 = wk.tile([128, FD], F16, tag="pA2")
        sc.copy(A2[:], psA2[:])
        yield
        psY = self.wave_ps(slot, "b")
        self.wave_mm(psY, A1, A2)
        yield
        y = wk.tile([128, FD], F16, tag="py")
        sc.copy(y[:], psY[:])
        yield

        acc = None
        for k, j in enumerate(range(m - 1, -1, -1)):
            pst = self.wave_ps(slot, "a" if k % 2 == 0 else "b")
            self.id_mm(pst, self.idws[pfx][(j, 2)], A2, start=True,
                       stop=False)
            self.id_mm(pst, self.idws[pfx][(j, 1)], A1,
                       stop=(acc is None))
            if acc is not None:
                yield
                self.wave_mm(pst, y, acc, start=False, stop=True)
            yield
            if j == 0:
                self.stt(v, out, self._bc(self.cIs[pfx][0], self.C),
                         1.0, pst[:])
            else:
                acc2 = wk.tile([128, FD], F16, tag="pacc")
                self.stt(v, acc2[:], self._bc(self.cIs[pfx][j], self.C),
                         1.0, pst[:])
                acc = acc2
            yield

    def prebuild_poly_consts(self, pfx, mono):
        deg = len(mono) - 1
        s = 3
        m = (deg + s) // s
        c = np.zeros(m * s)
        c[: deg + 1] = mono
        self.cIs[pfx] = {}
        self.idws[pfx] = {}
        for j in range(m):
            t = self.cst.tile([128, n], F16, tag=f"{pfx}cI{j}")
            self.nc.vector.tensor_scalar_mul(t[:], self.Ig[:], float(c[3 * j]))
            self.cIs[pfx][j] = t
            for r in (1, 2):
                self.idws[pfx][(j, r)] = self.idw_tile(
                    c[3 * j + r], f"{pfx}idw{j}_{r}")

    # ---------- tiny-matrix path (fp32) ----------
    def tiny_mm(self, lhsT, rhs, copy_to=None, tag="tmo"):
        nc = self.nc
        parts = lhsT.shape[0]
        pt = self.tiny_ps()
        nc.tensor.matmul(pt[0:64, :], lhsT[0:64, :], rhs[0:64, :],
                         start=True, stop=True)
        if parts == 128:
            nc.tensor.matmul(pt[64:128, :], lhsT[64:128, :], rhs[64:128, :],
                             start=True, stop=True)
        out = copy_to if copy_to is not None else self.tn.tile(
            [parts, n], F32, tag=tag)
        nc.scalar.copy(out[0:parts, :], pt[0:parts, :])
        return out

    def tiny_pair_mm(self, lhsT, rhs, tag="ps0a"):
        pt = self.tiny_ps(tag)
        self.nc.tensor.matmul(pt[0:64, :], lhsT[0:64, :], rhs[0:64, :],
                              start=True, stop=True)
        self.nc.tensor.matmul(pt[64:128, :], lhsT[64:128, :], rhs[64:128, :],
                              start=True, stop=True)
        return pt

    def tiny_cheb_gen(self, src, alpha, aff, out, pfx="", pstag="ps0a"):
        nc, v = self.nc, self.nc.vector
        s = alpha.shape[1]
        m = alpha.shape[0]
        beta, gamma = aff
        tn = self.tn
        Ah = tn.tile([128, n], F32, tag=pfx + "Ah")
        v.tensor_scalar_mul(Ah[:], src[:], float(beta))
        self.stt(v, Ah[:], self.Ig[:], gamma, Ah[:])
        yield
        T = [None, Ah]
        for r in range(2, s + 1):
            ps = self.tiny_pair_mm(Ah, T[r - 1], tag=pstag)
            Tr = tn.tile([128, n], F32, tag=pfx + f"T{r}")
            prev = self.Ig[:] if r == 2 else T[r - 2][:]
            self.stt(v, Tr[:], ps[:], 2.0, prev, MULT, SUB)
            T.append(Tr)
            yield
        yv = T[s]
        q = []
        for j in range(m):
            qj = tn.tile([128, n], F32, tag=pfx + f"q{j}")
            v.tensor_scalar_mul(qj[:], T[1][:], float(alpha[j, 1]))
            self.stt(v, qj[:], self.Ig[:], alpha[j, 0], qj[:])
            for r in range(2, s):
                self.stt(v, qj[:], T[r][:], alpha[j, r], qj[:])
            q.append(qj)
            yield
        b1, b2 = q[m - 1], None
        for j in range(m - 2, 0, -1):
            ps = self.tiny_pair_mm(yv, b1, tag=pstag)
            t = tn.tile([128, n], F32, tag=pfx + f"cl{j}")
            if b2 is None:
                self.stt(v, t[:], ps[:], 2.0, q[j][:], MULT, ADD)
                b1, b2 = t, b1
            else:
                self.stt(v, t[:], ps[:], 2.0, b2[:], MULT, SUB)
                t2 = tn.tile([128, n], F32, tag=pfx + f"cl2{j}")
                self.stt(v, t2[:], t[:], 1.0, q[j][:], MULT, ADD)
                b1, b2 = t2, b1
            yield
        ps = self.tiny_pair_mm(yv, b1, tag=pstag)
        if b2 is None:
            self.stt(v, out[:], ps[:], 1.0, q[0][:], MULT, ADD)
        else:
            t = tn.tile([128, n], F32, tag=pfx + "clF")
            self.stt(v, t[:], ps[:], 1.0, b2[:], MULT, SUB)
            self.stt(v, out[:], t[:], 1.0, q[0][:], MULT, ADD)
        yield

    def tiny_funcs(self, A_pair, rname, fnames, tagbase):
        outs = {}
        gens = []
        for i, fname in enumerate(fnames):
            alpha, aff = self.tiny_polys[(rname, fname)]
            o = self.tn.tile([128, n], F32, tag=tagbase + fname)
            gens.append(self.tiny_cheb_gen(
                A_pair, alpha, aff, o, pfx=f"ty{i}",
                pstag="ps0a" if i == 0 else "ps1a"))
            outs[fname] = o
        drive(gens, width=2)
        return outs

    def replicate(self, src64, tag="rep", dtype=F32):
        t = self.tn.tile([128, n], dtype, tag=tag)
        self.nc.vector.tensor_copy(t[0:64, :], src64[:])
        self.nc.vector.tensor_copy(t[64:128, :], src64[:])
        return t

    def allreduce64(self, acc_wide, width):
        nc, v = self.nc, self.nc.vector
        cur, w = acc_wide, width
        while w > n:
            nxt = self.tn.tile([128, w // 2], F32, tag=f"red{w}")
            v.tensor_add(nxt[:], cur[:, : w // 2], cur[:, w // 2:])
            cur, w = nxt, w // 2
        pt = self.tiny_ps()
        nc.tensor.matmul(pt[0:64, :], self.IIfold[:], cur[:, :],
                         start=True, stop=True)
        loc = self.tn.tile([64, n], F32, tag="arloc")
        nc.scalar.copy(loc[:], pt[0:64, :])
        bi = self.dp.tile([64, n], F32)
        bo = self.dp.tile([64, n], F32)
        nc.gpsimd.dma_start(bi[:], loc[:])
        nc.gpsimd.collective_compute(
            "AllReduce", ADD, replica_groups=[list(range(8))],
            ins=[bi.opt()], outs=[bo.opt()])
        res = self.tn.tile([64, n], F32, tag="arres")
        nc.gpsimd.dma_start(res[:], bo[:])
        return res

    def sqrt_refined(self, t, pfx):
        nc, v, sc = self.nc, self.nc.vector, self.nc.scalar
        u = self.tn.tile([1, 1], F32, tag=pfx + "u")
        sc.sqrt(u[:], t[:])
        for it in range(2):
            rec = self.tn.tile([1, 1], F32, tag=pfx + f"r{it}")
            v.reciprocal(rec[:], u[:])
            qt = self.tn.tile([1, 1], F32, tag=pfx + f"q{it}")
            v.tensor_mul(qt[:], t[:], rec[:])
            w = self.tn.tile([1, 1], F32, tag=pfx + f"w{it}")
            v.tensor_add(w[:], u[:], qt[:])
            u2 = self.tn.tile([1, 1], F32, tag=pfx + f"u{it}")
            v.tensor_scalar_mul(u2[:], w[:], 0.5)
            u = u2
        return u

    # ---------- phase chunk generators ----------
    def phaseA_gen(self, ci, x_in, Xp, xp_acc, gIbcS, y_out):
        nc, v = self.nc, self.nc.vector
        C, FD = self.C, self.FD
        st = self.stage
        slot = ci % 2
        xs = slice(ci * FD, (ci + 1) * FD)
        xt = self.io.tile([128, FD], F32, tag="xin")
        nc.sync.dma_start(xt[:], x_in.ap()[:, xs])
        yield
        xh = self.wk.tile([128, FD], F16, tag="xh")
        self.stt(v, xh[:], xt[:], self.sqrt_aff[0], self._bc(gIbcS, C))
        yield
        yield from self.poly_gen("S", xh, self.sqrt_mono, Xp[:, xs], slot)
        if st <= 1:
            ot = self.io.tile([128, FD], F32, tag="dbg")
            v.tensor_copy(ot[:], Xp[:, xs])
            nc.sync.dma_start(y_out.ap()[:, xs], ot[:])
        if ci == 0:
            v.tensor_copy(xp_acc[:], Xp[:, xs])
        else:
            v.tensor_add(xp_acc[:], xp_acc[:], Xp[:, xs])
        yield

    def phase_log_gen(self, ci, Xp, Rt16, aff, pfx, mono, out_tile, acc_tile,
                      gIbc, y_out, var_acc, vscr):
        """Shared by phases B and C. out_tile None -> scratch t1 tile."""
        nc, v, sc = self.nc, self.nc.vector, self.nc.scalar
        C, FD = self.C, self.FD
        st = self.stage
        slot = ci % 2
        xs = slice(ci * FD, (ci + 1) * FD)
        psU = self.wave_ps(slot, "a")
        self.wave_rep(psU, Xp, Rt16, lhs_off=ci * FD)
        yield
        U = self.wk.tile([128, FD], F16, tag="u")
        sc.copy(U[:], psU[:])
        yield
        psA = self.wave_ps(slot, "b")
        self.shared_mm(psA, Rt16, U)
        yield
        ah = self.wk.tile([128, FD], F16, tag="ah")
        self.stt(v, ah[:], psA[:], aff[0], self._bc(gIbc, C))
        yield
        if out_tile is None:
            t1 = self.wk.tile([128, FD], F16, tag="t1")
            tgt = t1[:]
        else:
            tgt = out_tile
        yield from self.poly_gen(pfx, ah, mono, tgt, slot)
        if acc_tile is not None:
            if st <= 3 and out_tile is None:
                ot = self.io.tile([128, FD], F32, tag="dbg")
                v.tensor_copy(ot[:], tgt)
                nc.sync.dma_start(y_out.ap()[:, xs], ot[:])
            if ci == 0:
                v.tensor_copy(acc_tile[:], tgt)
            else:
                v.tensor_add(acc_tile[:], acc_tile[:], tgt)
        if var_acc is not None:
            vred = self.tn.tile([128, 1], F32, tag="vred")
            sc.activation(vscr[:], tgt, mybir.ActivationFunctionType.Square,
                          accum_out=vred[:])
            v.tensor_add(var_acc[:], var_acc[:], vred[:])
        yield

    def phaseD1_gen(self, ci, Tres, Qt16):
        """Var-independent prefix: Hraw = Qt' T Qt, written into Tres."""
        nc, v, sc = self.nc, self.nc.vector, self.nc.scalar
        FD = self.FD
        slot = ci % 2
        xs = slice(ci * FD, (ci + 1) * FD)
        psU = self.wave_ps(slot, "a")
        self.wave_rep(psU, Tres, Qt16, lhs_off=ci * FD)
        yield
        U = self.wk.tile([128, FD], F16, tag="u")
        v.tensor_copy(U[:], psU[:])
        yield
        psH = self.wave_ps(slot, "b")
        self.shared_mm(psH, Qt16, U)
        yield
        sc.copy(Tres[:, xs], psH[:])
        yield

    def phaseD2_gen(self, ci, Tres, sr128, Pmt16, y_out):
        nc, v, sc = self.nc, self.nc.vector, self.nc.scalar
        C, FD = self.C, self.FD
        st = self.stage
        slot = ci % 2
        xs = slice(ci * FD, (ci + 1) * FD)
        hu = self.wk.tile([128, FD], F16, tag="ah")
        v.tensor_scalar_mul(hu[:], Tres[:, xs], sr128[:])
        yield
        E = self.wk.tile([128, FD], F16, tag="E")
        yield from self.poly_gen("E", hu, self.exp_mono, E[:], slot)
        if st <= 4.5:
            ot = self.io.tile([128, FD], F32, tag="dbg")
            v.tensor_copy(ot[:], E[:])
            nc.sync.dma_start(y_out.ap()[:, xs], ot[:])
            return
        psF = self.wave_ps(slot, "a")
        self.wave_rep(psF, E, Pmt16)
        yield
        Fu = self.wk.tile([128, FD], F16, tag="u")
        sc.copy(Fu[:], psF[:])
        yield
        psZ = self.wave_ps(slot, "b")
        self.shared_mm(psZ, Pmt16, Fu)
        yield
        Z = self.wk.tile([128, FD], F16, tag="Z")
        v.tensor_copy(Z[:], psZ[:])
        yield
        psO = self.wave_ps(slot, "a")
        self.wave_mm(psO, Z, Z)
        yield
        ot = self.io.tile([128, FD], F32, tag="yout")
        sc.copy(ot[:], psO[:])
        nc.sync.dma_start(y_out.ap()[:, xs], ot[:])
        yield

    # ---------- program ----------
    def build(self, *a, **k):
        from contextlib import ExitStack
        self._es = ExitStack()
        try:
            self._build(*a, **k)
        finally:
            self._es.close()

    def _build(self, x_in, m_in, w_in, shift_in, y_out):
        nc, tc = self.nc, self.tc
        v, g, sc = nc.vector, nc.gpsimd, nc.scalar
        C, FD, W = self.C, self.FD, self.W
        st = self.stage

        self.cst = self._es.enter_context(tc.tile_pool(name="cst", bufs=1))
        self.tn = self._es.enter_context(tc.tile_pool(name="tiny", bufs=2))
        self.wk = self._es.enter_context(tc.tile_pool(name="work", bufs=2))
        self.io = self._es.enter_context(tc.tile_pool(name="io", bufs=3))
        self.res = self._es.enter_context(tc.tile_pool(name="res", bufs=1))
        self.ps = self._es.enter_context(
            tc.tile_pool(name="ps", bufs=1, space="PSUM"))
        self.dp = self._es.enter_context(
            tc.tile_pool(name="dram", bufs=1, space="DRAM"))

        Ig = self.cst.tile([128, n], F32, tag="Ig")
        make_identity(nc, Ig[0:64, :])
        make_identity(nc, Ig[64:128, :])
        self.Ig = Ig
        I128 = self.cst.tile([128, 128], F32, tag="I128")
        make_identity(nc, I128[:, :])
        self.I128 = I128
        self.IIfold = self.cst.tile([128, n], F32, tag="IIfold")
        v.tensor_copy(self.IIfold[:], Ig[:])
        ones128 = self.cst.tile([128, 1], F32, tag="ones128")
        v.memset(ones128[:], 1.0)

        self.cIs = {}
        self.idws = {}
        self.prebuild_poly_consts("S", self.sqrt_mono)
        self.prebuild_poly_consts("L1", self.log1_mono)
        self.prebuild_poly_consts("L2", self.log2_mono)
        self.prebuild_poly_consts("E", self.exp_mono)
        gIbcS = self.scaled_identity(self.sqrt_aff[1], "gIbcS", F16)
        gIbc1 = self.scaled_identity(self.log1_aff[1], "gIbc1", F16)
        gIbc2 = self.scaled_identity(self.log2_aff[1], "gIbc2", F16)

        Xp = self.res.tile([128, W], F16, tag="Xp")
        Tres = self.res.tile([128, W], F16, tag="Tres")
        xp_acc = self.res.tile([128, FD], F32, tag="xpacc")
        t1_acc = self.res.tile([128, FD], F32, tag="t1acc")
        var_acc = self.res.tile([128, 1], F32, tag="vara")
        v.memset(var_acc[:], 0.0)
        vscr = self.res.tile([128, FD], F32, tag="vscr")

        with nc.named_scope("phaseA"):
            drive(self.phaseA_gen(ci, x_in, Xp, xp_acc, gIbcS, y_out)
                  for ci in range(self.n_chunks))
        if st <= 1:
            return

        # xp AllReduce first: the collective flies while tiny0 computes.
        with nc.named_scope("tinyA1"):
            xp_sum = self.allreduce64(xp_acc, FD)

        with nc.named_scope("tiny0"):
            M_sb = self.tn.tile([64, n], F32, tag="M")
            W_sb = self.tn.tile([64, n], F32, tag="Wt")
            shift_sb = self.tn.tile([1, 1], F32, tag="shift")
            nc.sync.dma_start(M_sb[:], m_in.ap())
            nc.sync.dma_start(W_sb[:], w_in.ap())
            nc.sync.dma_start(shift_sb[:], shift_in.ap())
            MW = self.tn.tile([128, n], F32, tag="MW")
            v.tensor_copy(MW[0:64, :], M_sb[:])
            v.tensor_copy(MW[64:128, :], W_sb[:])
            MWf = self.tiny_funcs(MW, "MW", ("sqrt", "rsqrt"), "fMW")
            Mh = self.tn.tile([64, n], F32, tag="Mh64")
            v.tensor_copy(Mh[:], MWf["sqrt"][0:64, :])
            Mnh = self.tn.tile([64, n], F32, tag="Mnh64")
            v.tensor_copy(Mnh[:], MWf["rsqrt"][0:64, :])
            Wh = self.tn.tile([64, n], F32, tag="Wh64")
            v.tensor_copy(Wh[:], MWf["sqrt"][64:128, :])
            Vt = self.tiny_mm(Wh, Mnh)
            Wc64 = self.tiny_mm(Mnh, Vt)
            WcP = self.replicate(Wc64)
            Wcf = self.tiny_funcs(WcP, "Wc", ("sqrt", "rsqrt"), "fWc")
            Wch = self.tn.tile([64, n], F32, tag="Wch64")
            v.tensor_copy(Wch[:], Wcf["sqrt"][0:64, :])
            Wcnh = self.tn.tile([64, n], F32, tag="Wcnh64")
            v.tensor_copy(Wcnh[:], Wcf["rsqrt"][0:64, :])
            Qt_raw = self.tiny_mm(Wh, Wcnh, tag="QtRaw")
            Pmt64 = self.tiny_mm(Wch, Mh, tag="Pmt64")
            Pmt_rep = self.replicate(Pmt64, tag="PmtRep")
            Pmt16 = self.tn.tile([128, n], F16, tag="Pmt16")
            v.tensor_copy(Pmt16[:], Pmt_rep[:])
            Qt_rep = self.replicate(Qt_raw, tag="QtRep")
            Qt16 = self.tn.tile([128, n], F16, tag="Qt16")
            v.tensor_copy(Qt16[:], Qt_rep[:])

        with nc.named_scope("tinyA"):
            Xpbar = self.tn.tile([64, n], F32, tag="xpbar")
            v.tensor_scalar_mul(Xpbar[:], xp_sum[:], 1.0 / self.B)
            V1 = self.tiny_mm(Xpbar, Mnh)
            G0 = self.tiny_mm(Mnh, V1)
            G0P = self.replicate(G0)
            G0f = self.tiny_funcs(G0P, "Gx", ("sqrt", "rsqrt"), "fG0")
            G0h = self.tn.tile([64, n], F32, tag="G0h64")
            v.tensor_copy(G0h[:], G0f["sqrt"][0:64, :])
            G0nh = self.tn.tile([64, n], F32, tag="G0nh64")
            v.tensor_copy(G0nh[:], G0f["rsqrt"][0:64, :])
            R1t64 = self.tiny_mm(Mnh, G0nh)
            R1t = self.replicate(R1t64, tag="R1tRep")
            R1t16 = self.tn.tile([128, n], F16, tag="R1t16")
            v.tensor_copy(R1t16[:], R1t[:])
        if st <= 2:
            ot = self.io.tile([128, n], F32, tag="dbg2")
            v.tensor_copy(ot[:], R1t[:])
            nc.sync.dma_start(y_out.ap()[:, 0:n], ot[:])
            return

        with nc.named_scope("phaseB"):
            drive(self.phase_log_gen(ci, Xp, R1t16, self.log1_aff, "L1",
                                     self.log1_mono, None, t1_acc, gIbc1,
                                     y_out, None, None)
                  for ci in range(self.n_chunks))
        if st <= 3:
            return
        with nc.named_scope("tinyB"):
            t1_sum = self.allreduce64(t1_acc, FD)
            Tbar = self.tn.tile([64, n], F32, tag="tbar")
            v.tensor_scalar_mul(Tbar[:], t1_sum[:], 1.0 / self.B)
            eT = self.tn.tile([64, n], F32, tag="eT")
            v.tensor_scalar_mul(eT[:], Ig[0:64, :],
                                self.expT_c[CFG["expT_deg"]])
            for k in range(CFG["expT_deg"] - 1, -1, -1):
                pt = self.tiny_ps()
                nc.tensor.matmul(pt[0:64, :], eT[:], Tbar[:],
                                 start=True, stop=True)
                eTn = self.tn.tile([64, n], F32, tag="eT")
                self.stt(v, eTn[:], Ig[0:64, :], self.expT_c[k], pt[0:64, :])
                eT = eTn
            V2 = self.tiny_mm(eT, G0h)
            G = self.tiny_mm(G0h, V2)
            GP = self.replicate(G)
            Gf = self.tiny_funcs(GP, "Gx", ("rsqrt",), "fG")
            mnh = self.tn.tile([64, n], F32, tag="mnh64")
            v.tensor_copy(mnh[:], Gf["rsqrt"][0:64, :])
            R2t64 = self.tiny_mm(Mnh, mnh)
            R2t = self.replicate(R2t64, tag="R2tRep")
            R2t16 = self.tn.tile([128, n], F16, tag="R2t16")
            v.tensor_copy(R2t16[:], R2t[:])

        with nc.named_scope("phaseC"):
            gens = []
            for ci in range(self.n_chunks):
                xs = slice(ci * FD, (ci + 1) * FD)
                gens.append(self.phase_log_gen(
                    ci, Xp, R2t16, self.log2_aff, "L2", self.log2_mono,
                    Tres[:, xs], None, gIbc2, y_out, var_acc, vscr))
            drive(gens)
        if st <= 3.5:
            for ci in range(self.n_chunks):
                xs = slice(ci * FD, (ci + 1) * FD)
                ot = self.io.tile([128, FD], F32, tag="dbg")
                v.tensor_copy(ot[:], Tres[:, xs])
                nc.sync.dma_start(y_out.ap()[:, xs], ot[:])
            return

        # var AllReduce launches before phase D1 so it hides behind it;
        # the 128-partition fold runs on the tensor engine (ones matmul).
        with nc.named_scope("tinyC1"):
            ptv = self.tiny_ps()
            nc.tensor.matmul(ptv[0:1, 0:1], var_acc[:, 0:1], ones128[:, 0:1],
                             start=True, stop=True)
            var_sb = self.tn.tile([1, 1], F32, tag="varsb")
            sc.copy(var_sb[:], ptv[0:1, 0:1])
            bi = self.dp.tile([1, 1], F32)
            bo = self.dp.tile([1, 1], F32)
            nc.gpsimd.dma_start(bi[:], var_sb[:])
            nc.gpsimd.collective_compute(
                "AllReduce", ADD, replica_groups=[list(range(8))],
                ins=[bi.opt()], outs=[bo.opt()])
            var_all = self.tn.tile([1, 1], F32, tag="varall")
            nc.gpsimd.dma_start(var_all[:], bo[:])

        with nc.named_scope("phaseD1"):
            drive(self.phaseD1_gen(ci, Tres, Qt16)
                  for ci in range(self.n_chunks))

        with nc.named_scope("tinyC"):
            tv = self.tn.tile([1, 1], F32, tag="tv")
            nc.vector.tensor_scalar(tv[:], var_all[:], 1.0 / self.B, EPS,
                                    MULT, ADD)
            uv = self.sqrt_refined(tv, "sva")
            rv = self.tn.tile([1, 1], F32, tag="rv")
            v.reciprocal(rv[:], uv[:])
            sv = self.tn.tile([1, 1], F32, tag="sv")
            v.tensor_mul(sv[:], rv[:], shift_sb[:])
            sr = self.tn.tile([1, 1], F32, tag="sr")
            v.tensor_scalar_mul(sr[:], sv[:], 1.0 / CFG["exp_r"])
            sr128 = self.tn.tile([128, 1], F32, tag="sr128")
            nc.gpsimd.partition_broadcast(sr128[:, :], sr[:, :])
        if st <= 4:
            sqv = self.sqrt_refined(sv, "svb")
            sq128 = self.tn.tile([128, 1], F32, tag="sq128")
            nc.gpsimd.partition_broadcast(sq128[:, :], sqv[:, :])
            Qst = self.tn.tile([128, n], F32, tag="Qst")
            nc.vector.tensor_scalar_mul(Qst[:], Qt_rep[:], sq128[:])
            ot = self.io.tile([128, n], F32, tag="dbg2")
            v.tensor_copy(ot[:], Qst[:])
            nc.sync.dma_start(y_out.ap()[:, 0:n], ot[:])
            return

        with nc.named_scope("phaseD2"):
            drive(self.phaseD2_gen(ci, Tres, sr128, Pmt16, y_out)
                  for ci in range(self.n_chunks))


def build_program(pairs_per_core, chunk_pairs, batch_total):
    nc = bacc.Bacc("TRN2", target_bir_lowering=False, debug=False,
                   num_devices=8)
    W = pairs_per_core * n
    x_in = nc.dram_tensor("x_in", [128, W], F32, kind="ExternalInput")
    m_in = nc.dram_tensor("m_in", [64, n], F32, kind="ExternalInput")
    w_in = nc.dram_tensor("w_in", [64, n], F32, kind="ExternalInput")
    shift_in = nc.dram_tensor("shift_in", [1, 1], F32, kind="ExternalInput")
    y_out = nc.dram_tensor("y_out", [128, W], F32, kind="ExternalOutput")
    with tile.TileContext(nc) as tc:
        em = Emit(nc, tc, pairs_per_core, chunk_pairs, batch_total)
        em.stage = float(os.environ.get("K_STAGE", "5"))
        em.build(x_in, m_in, w_in, shift_in, y_out)
    nc.compile()
    return nc


def pack_cores(Xb):
    B = Xb.shape[0]
    per = B // 8
    out = []
    for c in range(8):
        chunk = Xb[c * per:(c + 1) * per].reshape(per // 2, 2, n, n)
        arr = np.empty((128, (per // 2) * n), dtype=np.float32)
        arr[0:64] = chunk[:, 0].transpose(1, 0, 2).reshape(n, -1)
        arr[64:128] = chunk[:, 1].transpose(1, 0, 2).reshape(n, -1)
        out.append(np.ascontiguousarray(arr))
    return out


def unpack_cores(parts, B):
    per = B // 8
    Yb = np.empty((B, n, n), dtype=np.float32)
    for c in range(8):
        arr = parts[c]
        top = arr[0:64].reshape(n, per // 2, n).transpose(1, 0, 2)
        bot = arr[64:128].reshape(n, per // 2, n).transpose(1, 0, 2)
        chunk = np.stack([top, bot], axis=1).reshape(per, n, n)
        Yb[c * per:(c + 1) * per] = chunk
    return Yb


_PROG_CACHE = {}


def run_sharded(X, weight, M, shift, pairs_per_core, chunk_pairs, trace=False):
    B = X.shape[0]
    key = (pairs_per_core, chunk_pairs, B)
    if key not in _PROG_CACHE:
        _PROG_CACHE[key] = build_program(pairs_per_core, chunk_pairs, B)
    nc = _PROG_CACHE[key]
    xs = pack_cores(X.astype(np.float32))
    m_np = np.ascontiguousarray(M.astype(np.float32))
    w_np = np.ascontiguousarray(weight.astype(np.float32))
    s_np = np.array(shift, dtype=np.float32).reshape(1, 1)
    in_maps = [
        {"x_in": xs[c], "m_in": m_np, "w_in": w_np, "shift_in": s_np}
        for c in range(8)
    ]
    res = run_bass_kernel_spmd(nc, in_maps, core_ids=list(range(8)),
                               trace=trace)
    parts = [res.results[c]["y_out"] for c in range(8)]
    return unpack_cores(parts, B), res


def kernel(X, weight, M, shift):
    N, h = X.shape[0], X.shape[1]
    B = N * h
    Xb = np.asarray(X, dtype=np.float32).reshape(B, n, n)
    Yb, _ = run_sharded(Xb, np.asarray(weight), np.asarray(M),
                        np.asarray(shift), pairs_per_core=B // 16,
                        chunk_pairs=16)
    return Yb.reshape(X.shape).astype(np.float32)


# revision 4
# speedup vs baseline: 1.3389x; 1.3389x over previous
"""BatchNormSPD Trainium2 kernel (Bass/Tile), v5: trimmed polys, PE-lean.

Pipeline (per 64x64 SPD matrix, 4096 total, 512/core on 8 cores):
  Xp = sqrt(X)                  monomial-PS poly (deg 4, s=3) in fp16
  A1 = R1 Xp R1t, T1 = log(A1)  congruence + poly (deg 3, s=2)
  A2 = R2 Xp R2t, T  = log(A2)  congruence + poly (deg 4, s=3)
  H = Qt' T Qt, E = exp(sv*H)   poly (deg 3, s=2), sv folded into coeffs
  Z = Pmt' E Pmt, Y = Z @ Z
fp16 64x64 quadrant-pair matmuls (pair layout: top matrix in partitions
0-63, bottom in 64-127), fp32 PSUM. Final-block coefficient terms
(c1*A1 [+ c2*A2]) accumulate in PSUM via scaled-identity matmuls on the
tensor engine (emitted FIRST with start=True; accumulation onto
quadrant-started psum reads stale state, this order is exact). Top
Horner blocks are a single DVE stt (c*A + c*I, SBUF fp16). PSUM->SBUF
copies on Scalar; batch accumulators on GPSIMD (idle engine); the var
partition fold is a ones-vector matmul. The exp scale sv/r is folded
into runtime-scaled coefficient tiles so phase D2 needs no per-chunk
scaling op. A dummy warm-up AllReduce during phase A absorbs the first
collective's ~11us setup; the xp AllReduce launches before the tiny
M/W param chain, and the var AllReduce before phase D1, so both hide.
Chunks are emitted 2-way interleaved (generators); PSUM double-buffered
by chunk parity. Everything SBUF-resident; tiny shared-matrix path fp32.
"""
import math
import os

import numpy as np

import concourse.bacc as bacc
import concourse.tile as tile
from concourse import mybir
from concourse.bass_utils import run_bass_kernel_spmd
from concourse.masks import make_identity

F32 = mybir.dt.float32
F16 = mybir.dt.float16
MULT = mybir.AluOpType.mult
ADD = mybir.AluOpType.add
SUB = mybir.AluOpType.subtract

n = 64
EPS = 1e-5

CFG = dict(
    sqrt_ab=(0.44, 5.75), sqrt_deg=4, sqrt_s=3,
    log1_ab=(0.53, 2.15), log1_deg=3, log1_s=2,
    log2_ab=(0.56, 2.30), log2_deg=4, log2_s=3,
    exp_r=0.65, exp_deg=3, exp_s=2,
    expT_deg=6,
    tiny_tol=2e-6,
)

TINY_RANGES = dict(MW=(0.30, 3.30), Wc=(0.26, 3.45), Gx=(0.33, 3.72))


def cheb_coeffs(fn, a, b, ndeg):
    m = 8 * (ndeg + 1)
    theta = (np.arange(m) + 0.5) * np.pi / m
    x = np.cos(theta)
    xx = 0.5 * (b - a) * x + 0.5 * (b + a)
    fv = fn(xx)
    cc = np.zeros(ndeg + 1)
    for j in range(ndeg + 1):
        cc[j] = 2.0 / m * np.sum(fv * np.cos(j * theta))
    cc[0] *= 0.5
    return cc


def cheb_block_alpha(c, s):
    ndeg = len(c) - 1
    m = (ndeg + s) // s
    cc = np.zeros(m * s)
    cc[: ndeg + 1] = c
    alpha = np.zeros((m, s))
    for j in range(m - 1, 0, -1):
        alpha[j, 0] = cc[j * s]
        for r in range(1, s):
            val = 2 * cc[j * s + r]
            if j + 1 < m:
                val -= alpha[j + 1, s - r]
            alpha[j, r] = val
    alpha[0, 0] = cc[0]
    for r in range(1, s):
        alpha[0, r] = cc[r] - (0.5 * alpha[1, s - r] if m > 1 else 0.0)
    return alpha


def mono_poly(fn, a, b, deg):
    m = 8 * (deg + 2)
    u = np.cos((2 * np.arange(m) + 1) * np.pi / (2 * m))
    x = 0.5 * (b - a) * u + 0.5 * (b + a)
    V = np.polynomial.chebyshev.chebvander(u, deg)
    c, *_ = np.linalg.lstsq(V, fn(x), rcond=None)
    return np.polynomial.chebyshev.cheb2poly(c)


def drive(gens, width=2):
    """Round-robin-advance up to `width` generators; rolling admission."""
    active = []
    gens = list(gens)
    idx = 0
    while active or idx < len(gens):
        while len(active) < width and idx < len(gens):
            active.append(gens[idx])
            idx += 1
        for g in list(active):
            try:
                next(g)
            except StopIteration:
                active.remove(g)


class Emit:
    def __init__(self, nc, tc, pairs_per_core, chunk_pairs, batch_total):
        self.nc = nc
        self.tc = tc
        self.P = pairs_per_core
        self.C = chunk_pairs
        self.B = batch_total
        self.n_chunks = pairs_per_core // chunk_pairs
        self.FD = chunk_pairs * n
        self.W = pairs_per_core * n

        a, b = CFG["sqrt_ab"]
        self.sqrt_mono = mono_poly(np.sqrt, a, b, CFG["sqrt_deg"])
        self.sqrt_aff = (2.0 / (b - a), -(a + b) / (b - a))
        a, b = CFG["log1_ab"]
        self.log1_mono = mono_poly(np.log, a, b, CFG["log1_deg"])
        self.log1_aff = (2.0 / (b - a), -(a + b) / (b - a))
        a, b = CFG["log2_ab"]
        self.log2_mono = mono_poly(np.log, a, b, CFG["log2_deg"])
        self.log2_aff = (2.0 / (b - a), -(a + b) / (b - a))
        r = CFG["exp_r"]
        self.exp_mono = mono_poly(lambda u: np.exp(r * u), -1.0, 1.0,
                                  CFG["exp_deg"])
        self.expT_c = [1.0 / math.factorial(k)
                       for k in range(CFG["expT_deg"] + 1)]

        self.tiny_polys = {}
        for name, (a, b) in TINY_RANGES.items():
            for fname, fn in (("sqrt", np.sqrt),
                              ("rsqrt", lambda x: 1.0 / np.sqrt(x))):
                deg = None
                for d in range(8, 30):
                    c = cheb_coeffs(fn, a, b, d)
                    xs_ = np.linspace(a, b, 4001)
                    xh = (2 * xs_ - (a + b)) / (b - a)
                    err = np.abs(np.polynomial.chebyshev.chebval(xh, c)
                                 - fn(xs_)).max()
                    if err < CFG["tiny_tol"]:
                        deg = d
                        break
                assert deg is not None, (name, fname)
                self.tiny_polys[(name, fname)] = (
                    cheb_block_alpha(c, 5),
                    (2.0 / (b - a), -(a + b) / (b - a)))

    # ---------- helpers ----------
    def stt(self, eng, out, in0, scalar, in1, op0=MULT, op1=ADD):
        eng.scalar_tensor_tensor(out, in0, float(scalar), in1, op0, op1)

    def _bc(self, tiny, npairs):
        return tiny[:, None, :].to_broadcast((128, npairs, n))

    def scaled_identity(self, cval, tag, dtype=F32):
        t = self.cst.tile([128, n], dtype, tag=tag)
        self.nc.vector.tensor_scalar_mul(t[:], self.Ig[:], float(cval))
        return t

    def idw_tile(self, cval, tag):
        t = self.cst.tile([128, 128], F16, tag=tag)
        self.nc.vector.tensor_scalar_mul(t[:], self.I128[:], float(cval))
        return t

    def wave_ps(self, slot, which):
        return self.ps.tile([128, self.FD], F32, tag=f"ps{slot}{which}",
                            name=f"wps{slot}{which}")

    def tiny_ps(self, tag="ps0a"):
        return self.ps.tile([128, n], F32, tag=tag, name="tps")

    # ---------- wave matmuls (fp16 quadrant pairs) ----------
    def wave_mm(self, pt, lhsT, rhs, npairs=None, lhs_off=0, rhs_off=0,
                start=True, stop=True):
        nc = self.nc
        npairs = self.C if npairs is None else npairs
        for p in range(npairs):
            sl = slice(p * n, (p + 1) * n)
            ls = slice(lhs_off + p * n, lhs_off + (p + 1) * n)
            rs = slice(rhs_off + p * n, rhs_off + (p + 1) * n)
            nc.tensor.matmul(pt[0:64, sl], lhsT[0:64, ls], rhs[0:64, rs],
                             start=start, stop=stop, skip_group_check=True)
            nc.tensor.matmul(pt[64:128, sl], lhsT[64:128, ls], rhs[64:128, rs],
                             start=start, stop=stop, skip_group_check=True)

    def wave_rep(self, pt, lhsT, rep, npairs=None, lhs_off=0,
                 start=True, stop=True):
        nc = self.nc
        npairs = self.C if npairs is None else npairs
        for p in range(npairs):
            sl = slice(p * n, (p + 1) * n)
            ls = slice(lhs_off + p * n, lhs_off + (p + 1) * n)
            nc.tensor.matmul(pt[0:64, sl], lhsT[0:64, ls], rep[0:64, :],
                             start=start, stop=stop, skip_group_check=True)
            nc.tensor.matmul(pt[64:128, sl], lhsT[64:128, ls], rep[64:128, :],
                             start=start, stop=stop, skip_group_check=True)

    def shared_mm(self, pt, rep, rhs, npairs=None, rhs_off=0,
                  start=True, stop=True):
        nc = self.nc
        npairs = self.C if npairs is None else npairs
        width = npairs * n
        for h in range(0, width, 512):
            w = min(512, width - h)
            sl = slice(h, h + w)
            rs = slice(rhs_off + h, rhs_off + h + w)
            nc.tensor.matmul(pt[0:64, sl], rep[0:64, :], rhs[0:64, rs],
                             start=start, stop=stop, skip_group_check=True)
            nc.tensor.matmul(pt[64:128, sl], rep[64:128, :], rhs[64:128, rs],
                             start=start, stop=stop, skip_group_check=True)

    def id_mm(self, pt, coeff_tile, moving, npairs=None, moving_off=0,
              start=False, stop=False):
        nc = self.nc
        npairs = self.C if npairs is None else npairs
        width = npairs * n
        for h in range(0, width, 512):
            w = min(512, width - h)
            sl = slice(h, h + w)
            ms = slice(moving_off + h, moving_off + h + w)
            nc.tensor.matmul(pt[:, sl], coeff_tile[:, :], moving[:, ms],
                             start=start, stop=stop, skip_group_check=True)

    # ---------- big-batch polynomial (monomial PS), generator ----------
    def poly_gen(self, pfx, A1, mono, out, slot, s, off=0, rt_top=False):
        """Yield-granular evaluation of p(A1[:, off:off+FD]) into `out`.

        s = 2 or 3 (Paterson-Stockmeyer block size). m must be 2.
        Top Horner block = cH*A + cL*I as one DVE stt (rt_top=False) or
        via PSUM id_mm with runtime-scaled tiles (rt_top=True, pfx dicts
        hold runtime tiles). Final block: id_mm coeff terms + y@acc in
        PSUM, one stt out.
        """
        v, sc = self.nc.vector, self.nc.scalar
        deg = len(mono) - 1
        m = (deg + s) // s
        assert m == 2, (deg, s)
        c = np.zeros(m * s)
        c[: deg + 1] = mono
        FD = self.FD
        wk = self.wk
        A1v = A1[:, off:off + FD] if off or A1.shape[1] != FD else A1[:]

        psA2 = self.wave_ps(slot, "a")
        self.wave_mm(psA2, A1, A1, lhs_off=off, rhs_off=off)
        yield
        A2 = wk.tile([128, FD], F16, tag="pA2")
        sc.copy(A2[:], psA2[:])
        yield
        if s == 3:
            psY = self.wave_ps(slot, "b")
            self.wave_mm(psY, A1, A2, lhs_off=off)
            yield
            y = wk.tile([128, FD], F16, tag="py")
            sc.copy(y[:], psY[:])
            yield
            ymat = y
            blk0 = "a"
        else:
            ymat = A2
            blk0 = "b"

        # top block (j = 1): cH*A1 + cL*I
        acc = wk.tile([128, FD], F16, tag="pacc")
        if not rt_top:
            assert abs(c[s + 2] if s == 3 and len(c) > s + 2 else 0.0) == 0.0 \
                or s == 2, c
            if s == 3:
                assert c[5] == 0.0, c
            self.stt(v, acc[:], A1v, c[s + 1],
                     self._bc(self.cIs[pfx][1], self.C))
            yield
        else:
            pst = self.wave_ps(slot, blk0)
            self.id_mm(pst, self.idws[pfx][(1, 1)], A1, moving_off=off,
                       start=True, stop=True)
            yield
            self.stt(v, acc[:], self._bc(self.cIs[pfx][1], self.C),
                     1.0, pst[:])
            yield
            blk0 = "a" if blk0 == "b" else "b"

        # final block (j = 0): c1*A1 [+ c2*A2] + y@acc, then +c0*I via stt
        pst = self.wave_ps(slot, blk0 if rt_top else
                           ("a" if s == 3 else "b"))
        first = True
        for r in range(s - 1, 0, -1):
            key = (0, r)
            if key not in self.idws[pfx]:
                continue
            mv, mo = (A2, 0) if r == 2 else (A1, off)
            self.id_mm(pst, self.idws[pfx][key], mv, moving_off=mo,
                       start=first, stop=False)
            first = False
        yield
        self.wave_mm(pst, ymat, acc, start=first, stop=True)
        yield
        self.stt(v, out, self._bc(self.cIs[pfx][0], self.C), 1.0, pst[:])
        yield

    def prebuild_poly_consts(self, pfx, mono, s):
        deg = len(mono) - 1
        m = (deg + s) // s
        c = np.zeros(m * s)
        c[: deg + 1] = mono
        self.cIs[pfx] = {}
        self.idws[pfx] = {}
        for j in range(m):
            t = self.cst.tile([128, n], F16, tag=f"{pfx}cI{j}")
            self.nc.vector.tensor_scalar_mul(t[:], self.Ig[:], float(c[s * j]))
            self.cIs[pfx][j] = t
            for r in range(1, s):
                if c[s * j + r] != 0.0:
                    self.idws[pfx][(j, r)] = self.idw_tile(
                        c[s * j + r], f"{pfx}idw{j}_{r}")

    # ---------- tiny-matrix path (fp32) ----------
    def tiny_mm(self, lhsT, rhs, copy_to=None, tag="tmo"):
        nc = self.nc
        parts = lhsT.shape[0]
        pt = self.tiny_ps()
        nc.tensor.matmul(pt[0:64, :], lhsT[0:64, :], rhs[0:64, :],
                         start=True, stop=True)
        if parts == 128:
            nc.tensor.matmul(pt[64:128, :], lhsT[64:128, :], rhs[64:128, :],
                             start=True, stop=True)
        out = copy_to if copy_to is not None else self.tn.tile(
            [parts, n], F32, tag=tag)
        nc.scalar.copy(out[0:parts, :], pt[0:parts, :])
        return out

    def tiny_pair_mm(self, lhsT, rhs, tag="ps0a"):
        pt = self.tiny_ps(tag)
        self.nc.tensor.matmul(pt[0:64, :], lhsT[0:64, :], rhs[0:64, :],
                              start=True, stop=True)
        self.nc.tensor.matmul(pt[64:128, :], lhsT[64:128, :], rhs[64:128, :],
                              start=True, stop=True)
        return pt

    def tiny_cheb_gen(self, src, alpha, aff, out, pfx="", pstag="ps0a"):
        nc, v = self.nc, self.nc.vector
        s = alpha.shape[1]
        m = alpha.shape[0]
        beta, gamma = aff
        tn = self.tn
        Ah = tn.tile([128, n], F32, tag=pfx + "Ah")
        v.tensor_scalar_mul(Ah[:], src[:], float(beta))
        self.stt(v, Ah[:], self.Ig[:], gamma, Ah[:])
        yield
        T = [None, Ah]
        for r in range(2, s + 1):
            ps = self.tiny_pair_mm(Ah, T[r - 1], tag=pstag)
            Tr = tn.tile([128, n], F32, tag=pfx + f"T{r}")
            prev = self.Ig[:] if r == 2 else T[r - 2][:]
            self.stt(v, Tr[:], ps[:], 2.0, prev, MULT, SUB)
            T.append(Tr)
            yield
        yv = T[s]
        q = []
        for j in range(m):
            qj = tn.tile([128, n], F32, tag=pfx + f"q{j}")
            v.tensor_scalar_mul(qj[:], T[1][:], float(alpha[j, 1]))
            self.stt(v, qj[:], self.Ig[:], alpha[j, 0], qj[:])
            for r in range(2, s):
                self.stt(v, qj[:], T[r][:], alpha[j, r], qj[:])
            q.append(qj)
            yield
        b1, b2 = q[m - 1], None
        for j in range(m - 2, 0, -1):
            ps = self.tiny_pair_mm(yv, b1, tag=pstag)
            t = tn.tile([128, n], F32, tag=pfx + f"cl{j}")
            if b2 is None:
                self.stt(v, t[:], ps[:], 2.0, q[j][:], MULT, ADD)
                b1, b2 = t, b1
            else:
                self.stt(v, t[:], ps[:], 2.0, b2[:], MULT, SUB)
                t2 = tn.tile([128, n], F32, tag=pfx + f"cl2{j}")
                self.stt(v, t2[:], t[:], 1.0, q[j][:], MULT, ADD)
                b1, b2 = t2, b1
            yield
        ps = self.tiny_pair_mm(yv, b1, tag=pstag)
        if b2 is None:
            self.stt(v, out[:], ps[:], 1.0, q[0][:], MULT, ADD)
        else:
            t = tn.tile([128, n], F32, tag=pfx + "clF")
            self.stt(v, t[:], ps[:], 1.0, b2[:], MULT, SUB)
            self.stt(v, out[:], t[:], 1.0, q[0][:], MULT, ADD)
        yield

    def tiny_funcs(self, A_pair, rname, fnames, tagbase):
        outs = {}
        gens = []
        for i, fname in enumerate(fnames):
            alpha, aff = self.tiny_polys[(rname, fname)]
            o = self.tn.tile([128, n], F32, tag=tagbase + fname)
            gens.append(self.tiny_cheb_gen(
                A_pair, alpha, aff, o, pfx=f"ty{i}",
                pstag="ps0a" if i == 0 else "ps1a"))
            outs[fname] = o
        drive(gens, width=2)
        return outs

    def replicate(self, src64, tag="rep", dtype=F32):
        t = self.tn.tile([128, n], dtype, tag=tag)
        self.nc.vector.tensor_copy(t[0:64, :], src64[:])
        self.nc.vector.tensor_copy(t[64:128, :], src64[:])
        return t

    def allreduce64(self, acc_wide, width):
        nc, v = self.nc, self.nc.vector
        cur, w = acc_wide, width
        while w > n:
            nxt = self.tn.tile([128, w // 2], F32, tag=f"red{w}")
            v.tensor_add(nxt[:], cur[:, : w // 2], cur[:, w // 2:])
            cur, w = nxt, w // 2
        pt = self.tiny_ps()
        nc.tensor.matmul(pt[0:64, :], self.IIfold[:], cur[:, :],
                         start=True, stop=True)
        loc = self.tn.tile([64, n], F32, tag="arloc")
        nc.scalar.copy(loc[:], pt[0:64, :])
        bi = self.dp.tile([64, n], F32)
        bo = self.dp.tile([64, n], F32)
        nc.gpsimd.dma_start(bi[:], loc[:])
        nc.gpsimd.collective_compute(
            "AllReduce", ADD, replica_groups=[list(range(8))],
            ins=[bi.opt()], outs=[bo.opt()])
        res = self.tn.tile([64, n], F32, tag="arres")
        nc.gpsimd.dma_start(res[:], bo[:])
        return res

    def sqrt_refined(self, t, pfx):
        nc, v, sc = self.nc, self.nc.vector, self.nc.scalar
        u = self.tn.tile([1, 1], F32, tag=pfx + "u")
        sc.sqrt(u[:], t[:])
        for it in range(2):
            rec = self.tn.tile([1, 1], F32, tag=pfx + f"r{it}")
            v.reciprocal(rec[:], u[:])
            qt = self.tn.tile([1, 1], F32, tag=pfx + f"q{it}")
            v.tensor_mul(qt[:], t[:], rec[:])
            w = self.tn.tile([1, 1], F32, tag=pfx + f"w{it}")
            v.tensor_add(w[:], u[:], qt[:])
            u2 = self.tn.tile([1, 1], F32, tag=pfx + f"u{it}")
            v.tensor_scalar_mul(u2[:], w[:], 0.5)
            u = u2
        return u

    # ---------- phase chunk generators ----------
    def phaseA_gen(self, ci, x_in, Xp, xp_acc, gIbcS, y_out):
        nc, v, g = self.nc, self.nc.vector, self.nc.gpsimd
        C, FD = self.C, self.FD
        st = self.stage
        slot = ci % 2
        xs = slice(ci * FD, (ci + 1) * FD)
        xt = self.io.tile([128, FD], F32, tag="xin")
        nc.sync.dma_start(xt[:], x_in.ap()[:, xs])
        yield
        xh = self.wk.tile([128, FD], F16, tag="xh")
        self.stt(v, xh[:], xt[:], self.sqrt_aff[0], self._bc(gIbcS, C))
        yield
        yield from self.poly_gen("S", xh, self.sqrt_mono, Xp[:, xs], slot,
                                 CFG["sqrt_s"])
        if st <= 1:
            ot = self.io.tile([128, FD], F32, tag="dbg")
            v.tensor_copy(ot[:], Xp[:, xs])
            nc.sync.dma_start(y_out.ap()[:, xs], ot[:])
        if ci == 0:
            g.tensor_copy(xp_acc[:], Xp[:, xs])
        else:
            g.tensor_add(xp_acc[:], xp_acc[:], Xp[:, xs])
        yield

    def phase_log_gen(self, ci, Xp, Rt16, aff, pfx, mono, ps, out_tile,
                      acc_tile, gIbc, y_out, var_acc, vscr):
        """Shared by phases B and C. out_tile None -> scratch t1 tile."""
        nc, v, sc, g = self.nc, self.nc.vector, self.nc.scalar, self.nc.gpsimd
        C, FD = self.C, self.FD
        st = self.stage
        slot = ci % 2
        xs = slice(ci * FD, (ci + 1) * FD)
        psU = self.wave_ps(slot, "a")
        self.wave_rep(psU, Xp, Rt16, lhs_off=ci * FD)
        yield
        U = self.wk.tile([128, FD], F16, tag="u")
        sc.copy(U[:], psU[:])
        yield
        psA = self.wave_ps(slot, "b")
        self.shared_mm(psA, Rt16, U)
        yield
        ah = self.wk.tile([128, FD], F16, tag="ah")
        self.stt(v, ah[:], psA[:], aff[0], self._bc(gIbc, C))
        yield
        if out_tile is None:
            t1 = self.wk.tile([128, FD], F16, tag="t1")
            tgt = t1[:]
        else:
            tgt = out_tile
        yield from self.poly_gen(pfx, ah, mono, tgt, slot, ps)
        if acc_tile is not None:
            if st <= 3 and out_tile is None:
                ot = self.io.tile([128, FD], F32, tag="dbg")
                v.tensor_copy(ot[:], tgt)
                nc.sync.dma_start(y_out.ap()[:, xs], ot[:])
            if ci == 0:
                g.tensor_copy(acc_tile[:], tgt)
            else:
                g.tensor_add(acc_tile[:], acc_tile[:], tgt)
        if var_acc is not None:
            vred = self.tn.tile([128, 1], F32, tag="vred")
            sc.activation(vscr[:], tgt, mybir.ActivationFunctionType.Square,
                          accum_out=vred[:])
            v.tensor_add(var_acc[:], var_acc[:], vred[:])
        yield

    def phaseD1_gen(self, ci, Tres, Qt16):
        """Var-independent prefix: Hraw = Qt' T Qt, written into Tres."""
        nc, v, sc = self.nc, self.nc.vector, self.nc.scalar
        FD = self.FD
        slot = ci % 2
        xs = slice(ci * FD, (ci + 1) * FD)
        psU = self.wave_ps(slot, "a")
        self.wave_rep(psU, Tres, Qt16, lhs_off=ci * FD)
        yield
        U = self.wk.tile([128, FD], F16, tag="u")
        v.tensor_copy(U[:], psU[:])
        yield
        psH = self.wave_ps(slot, "b")
        self.shared_mm(psH, Qt16, U)
        yield
        sc.copy(Tres[:, xs], psH[:])
        yield

    def phaseD2_gen(self, ci, Tres, Pmt16, y_out):
        nc, v, sc = self.nc, self.nc.vector, self.nc.scalar
        C, FD = self.C, self.FD
        st = self.stage
        slot = ci % 2
        xs = slice(ci * FD, (ci + 1) * FD)
        E = self.wk.tile([128, FD], F16, tag="E")
        yield from self.poly_gen("Er", Tres, self.exp_mono, E[:], slot,
                                 CFG["exp_s"], off=ci * FD, rt_top=True)
        if st <= 4.5:
            ot = self.io.tile([128, FD], F32, tag="dbg")
            v.tensor_copy(ot[:], E[:])
            nc.sync.dma_start(y_out.ap()[:, xs], ot[:])
            return
        psF = self.wave_ps(slot, "b")
        self.wave_rep(psF, E, Pmt16)
        yield
        Fu = self.wk.tile([128, FD], F16, tag="u")
        sc.copy(Fu[:], psF[:])
        yield
        psZ = self.wave_ps(slot, "a")
        self.shared_mm(psZ, Pmt16, Fu)
        yield
        Z = self.wk.tile([128, FD], F16, tag="Z")
        v.tensor_copy(Z[:], psZ[:])
        yield
        psO = self.wave_ps(slot, "b")
        self.wave_mm(psO, Z, Z)
        yield
        ot = self.io.tile([128, FD], F32, tag="yout")
        sc.copy(ot[:], psO[:])
        nc.sync.dma_start(y_out.ap()[:, xs], ot[:])
        yield

    # ---------- program ----------
    def build(self, *a, **k):
        from contextlib import ExitStack
        self._es = ExitStack()
        try:
            self._build(*a, **k)
        finally:
            self._es.close()

    def _build(self, x_in, m_in, w_in, shift_in, y_out):
        nc, tc = self.nc, self.tc
        v, g, sc = nc.vector, nc.gpsimd, nc.scalar
        C, FD, W = self.C, self.FD, self.W
        st = self.stage

        self.cst = self._es.enter_context(tc.tile_pool(name="cst", bufs=1))
        self.tn = self._es.enter_context(tc.tile_pool(name="tiny", bufs=2))
        self.wk = self._es.enter_context(tc.tile_pool(name="work", bufs=3))
        self.io = self._es.enter_context(tc.tile_pool(name="io", bufs=3))
        self.res = self._es.enter_context(tc.tile_pool(name="res", bufs=1))
        self.ps = self._es.enter_context(
            tc.tile_pool(name="ps", bufs=1, space="PSUM"))
        self.dp = self._es.enter_context(
            tc.tile_pool(name="dram", bufs=1, space="DRAM"))

        Ig = self.cst.tile([128, n], F32, tag="Ig")
        make_identity(nc, Ig[0:64, :])
        make_identity(nc, Ig[64:128, :])
        self.Ig = Ig
        I128 = self.cst.tile([128, 128], F32, tag="I128")
        make_identity(nc, I128[:, :])
        self.I128 = I128
        self.IIfold = self.cst.tile([128, n], F32, tag="IIfold")
        v.tensor_copy(self.IIfold[:], Ig[:])
        ones128 = self.cst.tile([128, 1], F32, tag="ones128")
        v.memset(ones128[:], 1.0)

        self.cIs = {}
        self.idws = {}
        self.prebuild_poly_consts("S", self.sqrt_mono, CFG["sqrt_s"])
        self.prebuild_poly_consts("L1", self.log1_mono, CFG["log1_s"])
        self.prebuild_poly_consts("L2", self.log2_mono, CFG["log2_s"])
        self.prebuild_poly_consts("E", self.exp_mono, CFG["exp_s"])
        gIbcS = self.scaled_identity(self.sqrt_aff[1], "gIbcS", F16)
        gIbc1 = self.scaled_identity(self.log1_aff[1], "gIbc1", F16)
        gIbc2 = self.scaled_identity(self.log2_aff[1], "gIbc2", F16)

        Xp = self.res.tile([128, W], F16, tag="Xp")
        Tres = self.res.tile([128, W], F16, tag="Tres")
        xp_acc = self.res.tile([128, FD], F32, tag="xpacc")
        t1_acc = self.res.tile([128, FD], F32, tag="t1acc")
        var_acc = self.res.tile([128, 1], F32, tag="vara")
        v.memset(var_acc[:], 0.0)
        vscr = self.res.tile([128, FD], F32, tag="vscr")

        # warm-up AllReduce: absorbs the first collective's setup latency
        # while phase A runs; result unused.
        with nc.named_scope("warmcc"):
            dW = self.tn.tile([1, 1], F32, tag="dW")
            v.memset(dW[:], 0.0)
            dbi = self.dp.tile([1, 1], F32)
            dbo = self.dp.tile([1, 1], F32)
            nc.gpsimd.dma_start(dbi[:], dW[:])
            nc.gpsimd.collective_compute(
                "AllReduce", ADD, replica_groups=[list(range(8))],
                ins=[dbi.opt()], outs=[dbo.opt()])
            dres = self.tn.tile([1, 1], F32, tag="dWr")
            nc.gpsimd.dma_start(dres[:], dbo[:])

        with nc.named_scope("phaseA"):
            drive((self.phaseA_gen(ci, x_in, Xp, xp_acc, gIbcS, y_out)
                   for ci in range(self.n_chunks)), width=3)
        if st <= 1:
            return

        # xp AllReduce first: the collective flies while tiny0 computes.
        with nc.named_scope("tinyA1"):
            xp_sum = self.allreduce64(xp_acc, FD)

        with nc.named_scope("tiny0"):
            M_sb = self.tn.tile([64, n], F32, tag="M")
            W_sb = self.tn.tile([64, n], F32, tag="Wt")
            shift_sb = self.tn.tile([1, 1], F32, tag="shift")
            nc.sync.dma_start(M_sb[:], m_in.ap())
            nc.sync.dma_start(W_sb[:], w_in.ap())
            nc.sync.dma_start(shift_sb[:], shift_in.ap())
            MW = self.tn.tile([128, n], F32, tag="MW")
            v.tensor_copy(MW[0:64, :], M_sb[:])
            v.tensor_copy(MW[64:128, :], W_sb[:])
            MWf = self.tiny_funcs(MW, "MW", ("sqrt", "rsqrt"), "fMW")
            Mh = self.tn.tile([64, n], F32, tag="Mh64")
            v.tensor_copy(Mh[:], MWf["sqrt"][0:64, :])
            Mnh = self.tn.tile([64, n], F32, tag="Mnh64")
            v.tensor_copy(Mnh[:], MWf["rsqrt"][0:64, :])
            Wh = self.tn.tile([64, n], F32, tag="Wh64")
            v.tensor_copy(Wh[:], MWf["sqrt"][64:128, :])
            Vt = self.tiny_mm(Wh, Mnh)
            Wc64 = self.tiny_mm(Mnh, Vt)
            WcP = self.replicate(Wc64)
            Wcf = self.tiny_funcs(WcP, "Wc", ("sqrt", "rsqrt"), "fWc")
            Wch = self.tn.tile([64, n], F32, tag="Wch64")
            v.tensor_copy(Wch[:], Wcf["sqrt"][0:64, :])
            Wcnh = self.tn.tile([64, n], F32, tag="Wcnh64")
            v.tensor_copy(Wcnh[:], Wcf["rsqrt"][0:64, :])
            Qt_raw = self.tiny_mm(Wh, Wcnh, tag="QtRaw")
            Pmt64 = self.tiny_mm(Wch, Mh, tag="Pmt64")
            Pmt_rep = self.replicate(Pmt64, tag="PmtRep")
            Pmt16 = self.tn.tile([128, n], F16, tag="Pmt16")
            v.tensor_copy(Pmt16[:], Pmt_rep[:])
            Qt_rep = self.replicate(Qt_raw, tag="QtRep")
            Qt16 = self.tn.tile([128, n], F16, tag="Qt16")
            v.tensor_copy(Qt16[:], Qt_rep[:])

        with nc.named_scope("tinyA"):
            Xpbar = self.tn.tile([64, n], F32, tag="xpbar")
            v.tensor_scalar_mul(Xpbar[:], xp_sum[:], 1.0 / self.B)
            V1 = self.tiny_mm(Xpbar, Mnh)
            G0 = self.tiny_mm(Mnh, V1)
            G0P = self.replicate(G0)
            G0f = self.tiny_funcs(G0P, "Gx", ("sqrt", "rsqrt"), "fG0")
            G0h = self.tn.tile([64, n], F32, tag="G0h64")
            v.tensor_copy(G0h[:], G0f["sqrt"][0:64, :])
            G0nh = self.tn.tile([64, n], F32, tag="G0nh64")
            v.tensor_copy(G0nh[:], G0f["rsqrt"][0:64, :])
            R1t64 = self.tiny_mm(Mnh, G0nh)
            R1t = self.replicate(R1t64, tag="R1tRep")
            R1t16 = self.tn.tile([128, n], F16, tag="R1t16")
            v.tensor_copy(R1t16[:], R1t[:])
        if st <= 2:
            ot = self.io.tile([128, n], F32, tag="dbg2")
            v.tensor_copy(ot[:], R1t[:])
            nc.sync.dma_start(y_out.ap()[:, 0:n], ot[:])
            return

        with nc.named_scope("phaseB"):
            drive((self.phase_log_gen(ci, Xp, R1t16, self.log1_aff, "L1",
                                      self.log1_mono, CFG["log1_s"], None,
                                      t1_acc, gIbc1, y_out, None, None)
                   for ci in range(self.n_chunks)), width=3)
        if st <= 3:
            return
        with nc.named_scope("tinyB"):
            t1_sum = self.allreduce64(t1_acc, FD)
            Tbar = self.tn.tile([64, n], F32, tag="tbar")
            v.tensor_scalar_mul(Tbar[:], t1_sum[:], 1.0 / self.B)
            eT = self.tn.tile([64, n], F32, tag="eT")
            v.tensor_scalar_mul(eT[:], Ig[0:64, :],
                                self.expT_c[CFG["expT_deg"]])
            for k in range(CFG["expT_deg"] - 1, -1, -1):
                pt = self.tiny_ps()
                nc.tensor.matmul(pt[0:64, :], eT[:], Tbar[:],
                                 start=True, stop=True)
                eTn = self.tn.tile([64, n], F32, tag="eT")
                self.stt(v, eTn[:], Ig[0:64, :], self.expT_c[k], pt[0:64, :])
                eT = eTn
            V2 = self.tiny_mm(eT, G0h)
            G = self.tiny_mm(G0h, V2)
            GP = self.replicate(G)
            Gf = self.tiny_funcs(GP, "Gx", ("rsqrt",), "fG")
            mnh = self.tn.tile([64, n], F32, tag="mnh64")
            v.tensor_copy(mnh[:], Gf["rsqrt"][0:64, :])
            R2t64 = self.tiny_mm(Mnh, mnh)
            R2t = self.replicate(R2t64, tag="R2tRep")
            R2t16 = self.tn.tile([128, n], F16, tag="R2t16")
            v.tensor_copy(R2t16[:], R2t[:])

        with nc.named_scope("phaseC"):
            gens = []
            for ci in range(self.n_chunks):
                xs = slice(ci * FD, (ci + 1) * FD)
                gens.append(self.phase_log_gen(
                    ci, Xp, R2t16, self.log2_aff, "L2", self.log2_mono,
                    CFG["log2_s"], Tres[:, xs], None, gIbc2, y_out,
                    var_acc, vscr))
            drive(gens, width=3)
        if st <= 3.5:
            for ci in range(self.n_chunks):
                xs = slice(ci * FD, (ci + 1) * FD)
                ot = self.io.tile([128, FD], F32, tag="dbg")
                v.tensor_copy(ot[:], Tres[:, xs])
                nc.sync.dma_start(y_out.ap()[:, xs], ot[:])
            return

        # var AllReduce launches before phase D1 so it hides behind it;
        # the 128-partition fold runs on the tensor engine (ones matmul).
        with nc.named_scope("tinyC1"):
            ptv = self.tiny_ps()
            nc.tensor.matmul(ptv[0:1, 0:1], var_acc[:, 0:1], ones128[:, 0:1],
                             start=True, stop=True)
            var_sb = self.tn.tile([1, 1], F32, tag="varsb")
            sc.copy(var_sb[:], ptv[0:1, 0:1])
            bi = self.dp.tile([1, 1], F32)
            bo = self.dp.tile([1, 1], F32)
            nc.gpsimd.dma_start(bi[:], var_sb[:])
            nc.gpsimd.collective_compute(
                "AllReduce", ADD, replica_groups=[list(range(8))],
                ins=[bi.opt()], outs=[bo.opt()])
            var_all = self.tn.tile([1, 1], F32, tag="varall")
            nc.gpsimd.dma_start(var_all[:], bo[:])

        with nc.named_scope("phaseD1"):
            drive((self.phaseD1_gen(ci, Tres, Qt16)
                   for ci in range(self.n_chunks)), width=3)

        with nc.named_scope("tinyC"):
            tv = self.tn.tile([1, 1], F32, tag="tv")
            nc.vector.tensor_scalar(tv[:], var_all[:], 1.0 / self.B, EPS,
                                    MULT, ADD)
            uv = self.sqrt_refined(tv, "sva")
            rv = self.tn.tile([1, 1], F32, tag="rv")
            v.reciprocal(rv[:], uv[:])
            sv = self.tn.tile([1, 1], F32, tag="sv")
            v.tensor_mul(sv[:], rv[:], shift_sb[:])
            sr = self.tn.tile([1, 1], F32, tag="sr")
            v.tensor_scalar_mul(sr[:], sv[:], 1.0 / CFG["exp_r"])
            sr128 = self.tn.tile([128, 1], F32, tag="sr128")
            nc.gpsimd.partition_broadcast(sr128[:, :], sr[:, :])
            # fold t = sv/r into the exp coefficients: runtime tiles
            # scaled by t^k (k = s*j + r).
            t2_128 = self.tn.tile([128, 1], F32, tag="t2b")
            v.tensor_mul(t2_128[:], sr128[:], sr128[:])
            t3_128 = self.tn.tile([128, 1], F32, tag="t3b")
            v.tensor_mul(t3_128[:], t2_128[:], sr128[:])
            tpow = {1: sr128, 2: t2_128, 3: t3_128}
            es = CFG["exp_s"]
            rcI1 = self.tn.tile([128, n], F16, tag="ErcI1")
            v.tensor_scalar_mul(rcI1[:], self.cIs["E"][1][:], tpow[es][:])
            self.cIs["Er"] = {0: self.cIs["E"][0], 1: rcI1}
            self.idws["Er"] = {}
            for (j, r), base in self.idws["E"].items():
                k = es * j + r
                rt = self.tn.tile([128, 128], F16, tag=f"Eridw{j}_{r}")
                v.tensor_scalar_mul(rt[:], base[:], tpow[k][:])
                self.idws["Er"][(j, r)] = rt
        if st <= 4:
            sqv = self.sqrt_refined(sv, "svb")
            sq128 = self.tn.tile([128, 1], F32, tag="sq128")
            nc.gpsimd.partition_broadcast(sq128[:, :], sqv[:, :])
            Qst = self.tn.tile([128, n], F32, tag="Qst")
            nc.vector.tensor_scalar_mul(Qst[:], Qt_rep[:], sq128[:])
            ot = self.io.tile([128, n], F32, tag="dbg2")
            v.tensor_copy(ot[:], Qst[:])
            nc.sync.dma_start(y_out.ap()[:, 0:n], ot[:])
            return

        with nc.named_scope("phaseD2"):
            drive((self.phaseD2_gen(ci, Tres, Pmt16, y_out)
                   for ci in range(self.n_chunks)), width=3)


def build_program(pairs_per_core, chunk_pairs, batch_total):
    nc = bacc.Bacc("TRN2", target_bir_lowering=False, debug=False,
                   num_devices=8)
    W = pairs_per_core * n
    x_in = nc.dram_tensor("x_in", [128, W], F32, kind="ExternalInput")
    m_in = nc.dram_tensor("m_in", [64, n], F32, kind="ExternalInput")
    w_in = nc.dram_tensor("w_in", [64, n], F32, kind="ExternalInput")
    shift_in = nc.dram_tensor("shift_in", [1, 1], F32, kind="ExternalInput")
    y_out = nc.dram_tensor("y_out", [128, W], F32, kind="ExternalOutput")
    with tile.TileContext(nc) as tc:
        em = Emit(nc, tc, pairs_per_core, chunk_pairs, batch_total)
        em.stage = float(os.environ.get("K_STAGE", "5"))
        em.build(x_in, m_in, w_in, shift_in, y_out)
    nc.compile()
    return nc


def pack_cores(Xb):
    B = Xb.shape[0]
    per = B // 8
    out = []
    for c in range(8):
        chunk = Xb[c * per:(c + 1) * per].reshape(per // 2, 2, n, n)
        arr = np.empty((128, (per // 2) * n), dtype=np.float32)
        arr[0:64] = chunk[:, 0].transpose(1, 0, 2).reshape(n, -1)
        arr[64:128] = chunk[:, 1].transpose(1, 0, 2).reshape(n, -1)
        out.append(np.ascontiguousarray(arr))
    return out


def unpack_cores(parts, B):
    per = B // 8
    Yb = np.empty((B, n, n), dtype=np.float32)
    for c in range(8):
        arr = parts[c]
        top = arr[0:64].reshape(n, per // 2, n).transpose(1, 0, 2)
        bot = arr[64:128].reshape(n, per // 2, n).transpose(1, 0, 2)
        chunk = np.stack([top, bot], axis=1).reshape(per, n, n)
        Yb[c * per:(c + 1) * per] = chunk
    return Yb


_PROG_CACHE = {}


def run_sharded(X, weight, M, shift, pairs_per_core, chunk_pairs, trace=False):
    B = X.shape[0]
    key = (pairs_per_core, chunk_pairs, B)
    if key not in _PROG_CACHE:
        _PROG_CACHE[key] = build_program(pairs_per_core, chunk_pairs, B)
    nc = _PROG_CACHE[key]
    xs = pack_cores(X.astype(np.float32))
    m_np = np.ascontiguousarray(M.astype(np.float32))
    w_np = np.ascontiguousarray(weight.astype(np.float32))
    s_np = np.array(shift, dtype=np.float32).reshape(1, 1)
    in_maps = [
        {"x_in": xs[c], "m_in": m_np, "w_in": w_np, "shift_in": s_np}
        for c in range(8)
    ]
    res = run_bass_kernel_spmd(nc, in_maps, core_ids=list(range(8)),
                               trace=trace)
    parts = [res.results[c]["y_out"] for c in range(8)]
    return unpack_cores(parts, B), res


def kernel(X, weight, M, shift):
    N, h = X.shape[0], X.shape[1]
    B = N * h
    Xb = np.asarray(X, dtype=np.float32).reshape(B, n, n)
    Yb, _ = run_sharded(Xb, np.asarray(weight), np.asarray(M),
                        np.asarray(shift), pairs_per_core=B // 16,
                        chunk_pairs=16)
    return Yb.reshape(X.shape).astype(np.float32)
